# revision 1
# baseline (speedup 1.0000x reference)
"""AttentionBlock (GroupNorm(32) + 1-head self-attention + proj + residual) on 8 trn2 cores.

Data-parallel over batch: each of the 8 NeuronCores processes 2 of the 16 images.
All matmuls run in float32r (full PE rate, ~1e-4 relative precision).

Layout strategy (per image, c=512 channels, n=1024 positions):
  - x, xn, q, k in (channel, position) layout: [128, 4, 1024] / 4x[128, 1024] tiles.
  - v computed directly transposed (position, channel): 8 tiles of [128, 512],
    so the attention contraction over positions j needs no explicit transposes.
  - scores computed transposed: sT[j, i] = k[:,j]. q[:,i]  (softmax scale folded
    into Wq host-side). Softmax over j = exp (no max subtraction; scores are
    ~N(0,1) by construction) + column sums l[i] via an all-ones stationary
    matmul, which also broadcasts l across partitions for free.
  - GroupNorm stats via indicator matmuls (indicator pre-scaled by 1/(16*1024)
    so the sums come out as means); rstd via Newton rsqrt on DVE and 1/l via
    reciprocal_approx_fast, so ScalarE only ever needs one ACT table set
    (exp/copy/identity) -- no table switches.
  - attn@V accumulates over j; normalization by 1/l is folded into the
    PSUM->SBUF copy; proj + residual-add fused into the final DVE add.
  - DMAs are batched; per-phase emission order software-pipelines the two
    images (image 2's GroupNorm chain runs under image 1's attention).
"""

import ml_dtypes
import numpy as np

import concourse.bacc as bacc
import concourse.tile as tile
import concourse.mybir as mybir
from concourse.bass_utils import run_bass_kernel_spmd

F32 = mybir.dt.float32
F32R = mybir.dt.float32r
BF16 = mybir.dt.bfloat16
I32 = mybir.dt.int32
AF = mybir.ActivationFunctionType
ALU = mybir.AluOpType
AX = mybir.AxisListType

B, C, H, W = 16, 512, 32, 32
N = H * W                 # 1024 positions
NCORES = 8
BPC = B // NCORES         # 2 images per core
G = 32                    # groupnorm groups
GS = C // G               # 16 channels per group
CT = C // 128             # 4 channel tiles
NT = N // 128             # 8 position tiles
NH = N // 512             # 2 free-dim halves
EPS = 1e-5
SCALE = float(C) ** -0.5  # single head, head_dim = C
MAGIC = 0x5F3759DF        # Newton-rsqrt seed constant

_cache: dict = {}

# attention-path matmul dtype: "f32r" (default) or "bf16"
ATTN_DT = "f32r"


def _build(repeat: int = 1, zero_qk_bias: bool = True, loop_iters: int = 0,
           attn_dtype=F32R):
    MMDT = attn_dtype
    nc = bacc.Bacc("TRN2", target_bir_lowering=False, num_devices=NCORES)

    # f32r DRAM tensors so nc.sync DMA needs no cast (numpy side is float32).
    x_d = nc.dram_tensor("x", [BPC, C, N], F32R, kind="ExternalInput")
    wq_d = nc.dram_tensor("wq", [C, C], MMDT, kind="ExternalInput")    # (c, o) = (scale*Wq).T
    wk_d = nc.dram_tensor("wk", [C, C], MMDT, kind="ExternalInput")    # Wk.T
    wv_d = nc.dram_tensor("wv", [C, C], MMDT, kind="ExternalInput")    # Wv.T
    wp_d = nc.dram_tensor("wp", [C, C], MMDT, kind="ExternalInput")    # Wp.T
    ind_d = nc.dram_tensor("ind", [C, G], F32R, kind="ExternalInput")  # (1/(16*1024)) iff c//16==g
    bind_d = nc.dram_tensor("bind", [G, C], F32R, kind="ExternalInput")  # 0/1 indicator.T
    ones_d = nc.dram_tensor("onesm", [128, 128], MMDT, kind="ExternalInput")
    # consts: [gnsc | gnbi | bq | bk], each (128, CT)
    consts_d = nc.dram_tensor("consts", [128, 4 * CT], F32, kind="ExternalInput")
    out_d = nc.dram_tensor("out", [BPC, C, N], F32, kind="ExternalOutput")

    with tile.TileContext(nc) as tc:
        with (
            tc.tile_pool(name="wpool", bufs=1) as wp_,
            tc.tile_pool(name="xpool", bufs=2) as xpool,
            tc.tile_pool(name="xnpool", bufs=CT) as xnpool,
            tc.tile_pool(name="sqpool", bufs=1) as sqpool,
            tc.tile_pool(name="qpool", bufs=CT) as qpool,
            tc.tile_pool(name="kpool", bufs=CT) as kpool,
            tc.tile_pool(name="vpool", bufs=NT) as vpool,
            tc.tile_pool(name="epool", bufs=NT) as epool,
            tc.tile_pool(name="fpool", bufs=1) as fpool,
            tc.tile_pool(name="rpool", bufs=2) as rpool,
            tc.tile_pool(name="spool", bufs=2) as spool,
            tc.tile_pool(name="psA", bufs=3, space="PSUM") as psA,
            tc.tile_pool(name="psB", bufs=2, space="PSUM") as psB,
        ):
            # ---- persistent constants / weights (batched single DMAs) ----
            wq_all = wp_.tile([128, CT, C], MMDT, tag="wq", name="wq")
            wk_all = wp_.tile([128, CT, C], MMDT, tag="wk", name="wk")
            wv_all = wp_.tile([128, CT, C], MMDT, tag="wv", name="wv")
            wp_all = wp_.tile([128, CT, C], MMDT, tag="wp", name="wp")
            ind_all = wp_.tile([128, CT, G], F32R, tag="ind", name="ind")
            bind_all = wp_.tile([G, CT, 128], F32R, tag="bind", name="bind")
            ones_sb = wp_.tile([128, 128], MMDT, tag="ones", name="ones")
            consts_sb = wp_.tile([128, 4 * CT], F32, tag="consts", name="consts")
            magic_sb = wp_.tile([128, 1], I32, tag="magic", name="magic")
            nc.vector.memset(magic_sb, MAGIC)
            gnsc_sb = consts_sb[:, 0 * CT:1 * CT]
            gnbi_sb = consts_sb[:, 1 * CT:2 * CT]
            bq_sb = consts_sb[:, 2 * CT:3 * CT]
            bk_sb = consts_sb[:, 3 * CT:4 * CT]

            def part(dram2d):
                # (T*128, F) -> [128, T, F]
                return dram2d.rearrange("(t p) f -> p t f", p=128)

            # issue order matters: the first stats matmul needs only
            # ind + the first half of x tile 0 -- land those first
            x0 = xpool.tile([128, CT, N], F32R, tag="x", name="x")
            nc.sync.dma_start(out=x0[:, 0, 0:512], in_=x_d[0, 0:128, 0:512])
            nc.sync.dma_start(out=ind_all, in_=part(ind_d[:, :]))
            nc.sync.dma_start(out=x0[:, 0, 512:1024], in_=x_d[0, 0:128, 512:1024])
            for t in range(1, CT):
                nc.sync.dma_start(
                    out=x0[:, t, :], in_=x_d[0, 128 * t:128 * (t + 1), :]
                )
            nc.sync.dma_start(out=consts_sb, in_=consts_d[:, :])
            nc.sync.dma_start(
                out=bind_all, in_=bind_d.rearrange("g (t p) -> g t p", p=128)
            )
            nc.sync.dma_start(out=wq_all, in_=part(wq_d[:, :]))
            nc.sync.dma_start(out=wk_all, in_=part(wk_d[:, :]))
            nc.sync.dma_start(out=wv_all, in_=part(wv_d[:, :]))
            nc.sync.dma_start(out=ones_sb, in_=ones_d[:, :])
            nc.sync.dma_start(out=wp_all, in_=part(wp_d[:, :]))

            halves = [slice(0, 512), slice(512, 1024)]

            def emit_gn(img, x_pre=None):
                """Load x, GroupNorm stats + chain, xn apply.
                Returns state dict for later phases."""
                if x_pre is not None:
                    x_all = x_pre
                else:
                    x_all = xpool.tile([128, CT, N], F32R, tag="x", name="x")
                    for t in range(CT):
                        nc.sync.dma_start(
                            out=x_all[:, t, :],
                            in_=x_d[img, 128 * t:128 * (t + 1), :],
                        )
                xt = [x_all[:, t, :] for t in range(CT)]

                # x^2
                xsq_all = sqpool.tile([128, CT, N], F32R, tag="sq", name="sq")
                xf = x_all.bitcast(F32)
                for t in range(CT):
                    nc.vector.tensor_mul(xsq_all[:, t, :], xf[:, t, :], xf[:, t, :])

                # group means via (1/(16*1024))-scaled indicator matmuls
                sum_ps = psB.tile([G, 512], F32, tag="psB", name="psB")
                sum_ps2 = psB.tile([G, 512], F32, tag="psB", name="psB")
                sq_ps = psB.tile([G, 512], F32, tag="psB", name="psB")
                sq_ps2 = psB.tile([G, 512], F32, tag="psB", name="psB")
                for t in range(CT):
                    for h, ps in ((0, sum_ps), (1, sum_ps2)):
                        nc.tensor.matmul(
                            ps[:, :], ind_all[:, t, :], xt[t][:, halves[h]],
                            start=(t == 0), stop=(t == CT - 1),
                        )
                for t in range(CT):
                    for h, ps in ((0, sq_ps), (1, sq_ps2)):
                        nc.tensor.matmul(
                            ps[:, :], ind_all[:, t, :], xsq_all[:, t, halves[h]],
                            start=(t == 0), stop=(t == CT - 1),
                        )

                # reduce to [mean_h0, mean_h1, e2_h0, e2_h1] (already scaled)
                st4 = spool.tile([G, 4], F32, tag="st4", name="st4")
                recipbc = rpool.tile([128, N], F32, tag="rbc", name="rbc")
                nc.vector.reduce_sum(out=st4[:, 0:1], in_=sum_ps[:, :], axis=AX.X)
                nc.vector.reduce_sum(out=st4[:, 1:2], in_=sum_ps2[:, :], axis=AX.X)
                # recipbc is fully overwritten much later; use as throwaway ACT
                # output (we only want accum_out).
                nc.scalar.activation(out=recipbc[0:G, 0:512], in_=sq_ps[:, :],
                                     func=AF.Copy, accum_out=st4[:, 2:3])
                nc.scalar.activation(out=recipbc[0:G, 512:1024], in_=sq_ps2[:, :],
                                     func=AF.Copy, accum_out=st4[:, 3:4])
                # e12 = [mean, E[x^2]]
                e12 = spool.tile([G, 2], F32, tag="e12", name="e12")
                st4_v = st4.rearrange("g (s h) -> g s h", h=2)
                nc.vector.tensor_add(e12[:, :], st4_v[:, :, 0], st4_v[:, :, 1])
                # vpe = E2 + eps - mean^2
                vpe = spool.tile([G, 1], F32, tag="vpe", name="vpe")
                msq = spool.tile([G, 1], F32, tag="msq", name="msq")
                nc.vector.tensor_mul(msq[:, :], e12[:, 0:1], e12[:, 0:1])
                nc.vector.scalar_tensor_tensor(
                    out=vpe[:, :], in0=e12[:, 1:2], scalar=EPS, in1=msq[:, :],
                    op0=ALU.add, op1=ALU.subtract,
                )
                # rstd = 1/sqrt(vpe): bit-trick seed + 2 Newton iterations
                sh_t = spool.tile([G, 1], I32, tag="sh", name="sh")
                nc.vector.tensor_scalar(
                    out=sh_t[:, :], in0=vpe.bitcast(I32)[:, :], scalar1=1,
                    scalar2=None, op0=ALU.logical_shift_right,
                )
                seed = spool.tile([G, 1], I32, tag="seed", name="seed")
                nc.vector.scalar_tensor_tensor(
                    out=seed[:, :], in0=magic_sb[:G, :], scalar=0, in1=sh_t[:, :],
                    op0=ALU.bypass, op1=ALU.subtract,
                )
                y = seed.bitcast(F32)
                for it in range(2):
                    t1 = spool.tile([G, 1], F32, tag=f"nr{it}", name=f"nr{it}")
                    nc.vector.tensor_mul(t1[:, :], y[:, :], y[:, :])
                    nc.vector.tensor_mul(t1[:, :], t1[:, :], vpe[:, :])
                    nc.vector.tensor_scalar(
                        out=t1[:, :], in0=t1[:, :], scalar1=-0.5, scalar2=1.5,
                        op0=ALU.mult, op1=ALU.add,
                    )
                    y2 = spool.tile([G, 1], F32, tag=f"y{it}", name=f"y{it}")
                    nc.vector.tensor_mul(y2[:, :], y[:, :], t1[:, :])
                    y = y2
                # stats2 = [rstd, mean] (f32r for the broadcast matmul)
                stats2 = spool.tile([G, 2], F32R, tag="st2", name="st2")
                nc.vector.tensor_copy(stats2[:, 0:1], y[:, :])
                nc.vector.tensor_copy(stats2[:, 1:2], e12[:, 0:1])

                # broadcast to channels; a = gnsc*rstd, b = gnbi - mean*a
                bc_ps = psB.tile([128, 2 * CT], F32, tag="psB", name="psB")
                for t in range(CT):
                    nc.tensor.matmul(
                        bc_ps[:, 2 * t:2 * t + 2], bind_all[:, t, :], stats2[:, :],
                        start=True, stop=True,
                    )
                bc_sb = spool.tile([128, 2 * CT], F32, tag="bc", name="bc")
                nc.vector.tensor_copy(bc_sb[:, :], bc_ps[:, :])
                bc_v = bc_sb.rearrange("p (t s) -> p t s", s=2)
                a_all = spool.tile([128, CT], F32, tag="aall", name="aall")
                b_all = spool.tile([128, CT], F32, tag="ball", name="ball")
                nc.vector.tensor_mul(a_all[:, :], gnsc_sb, bc_v[:, :, 0])
                nc.vector.scalar_tensor_tensor(
                    out=b_all[:, :], in0=bc_v[:, :, 1], scalar=-1.0, in1=a_all[:, :],
                    op0=ALU.mult, op1=ALU.mult,
                )
                nc.vector.tensor_add(b_all[:, :], b_all[:, :], gnbi_sb)

                # xn = a*x + b (ACT; Identity is in every table set)
                xn = [xnpool.tile([128, N], MMDT, tag="xn", name="xn") for _ in range(CT)]
                for t in range(CT):
                    nc.scalar.activation(
                        out=xn[t][:, :], in_=xt[t].bitcast(F32), func=AF.Identity,
                        scale=a_all[:, t:t + 1], bias=b_all[:, t:t + 1],
                    )
                return {"xt": xt, "xn": xn, "recipbc": recipbc, "img": img}

            def emit_qkv(s):
                xn = s["xn"]
                q = [qpool.tile([128, N], MMDT, tag="q", name="q") for _ in range(CT)]
                k = [kpool.tile([128, N], MMDT, tag="k", name="k") for _ in range(CT)]
                for d in range(CT):
                    ds_ = slice(128 * d, 128 * (d + 1))
                    qps = psA.tile([128, N], F32, tag="psA", name="psA")
                    for t in range(CT):
                        for h in range(NH):
                            nc.tensor.matmul(
                                qps[:, halves[h]], wq_all[:, t, ds_], xn[t][:, halves[h]],
                                start=(t == 0), stop=(t == CT - 1),
                            )
                    if zero_qk_bias:
                        nc.scalar.copy(out=q[d][:, :], in_=qps[:, :])
                    else:
                        nc.scalar.activation(out=q[d][:, :], in_=qps[:, :],
                                             func=AF.Identity,
                                             bias=bq_sb[:, d:d + 1], scale=1.0)
                    kps = psA.tile([128, N], F32, tag="psA", name="psA")
                    for t in range(CT):
                        for h in range(NH):
                            nc.tensor.matmul(
                                kps[:, halves[h]], wk_all[:, t, ds_], xn[t][:, halves[h]],
                                start=(t == 0), stop=(t == CT - 1),
                            )
                    if zero_qk_bias:
                        nc.scalar.copy(out=k[d][:, :], in_=kps[:, :])
                    else:
                        nc.scalar.activation(out=k[d][:, :], in_=kps[:, :],
                                             func=AF.Identity,
                                             bias=bk_sb[:, d:d + 1], scale=1.0)
                # vT[n, o']: lhsT = xn[:, n-slice], rhs = wv
                vT = [vpool.tile([128, C], MMDT, tag="v", name="v") for _ in range(NT)]
                for n in range(NT):
                    ns = slice(128 * n, 128 * (n + 1))
                    vps = psB.tile([128, 512], F32, tag="psB", name="psB")
                    for t in range(CT):
                        nc.tensor.matmul(
                            vps[:, :], xn[t][:, ns], wv_all[:, t, :],
                            start=(t == 0), stop=(t == CT - 1),
                        )
                    nc.vector.tensor_copy(vT[n][:, :], vps[:, :])
                s["q"], s["k"], s["vT"] = q, k, vT

            def emit_att(s):
                q, k, vT, recipbc = s["q"], s["k"], s["vT"], s["recipbc"]
                expT = [epool.tile([128, N], MMDT, tag="e", name="e") for _ in range(NT)]
                for j in range(NT):
                    js = slice(128 * j, 128 * (j + 1))
                    sps = psA.tile([128, N], F32, tag="psA", name="psA")
                    for d in range(CT):
                        for h in range(NH):
                            nc.tensor.matmul(
                                sps[:, halves[h]], k[d][:, js], q[d][:, halves[h]],
                                start=(d == 0), stop=(d == CT - 1),
                            )
                    nc.scalar.activation(out=expT[j][:, :], in_=sps[:, :], func=AF.Exp)

                # l[i] = sum_j exp, broadcast across partitions; 1/l on DVE
                lps = psA.tile([128, N], F32, tag="psA", name="psA")
                for j in range(NT):
                    for h in range(NH):
                        nc.tensor.matmul(
                            lps[:, halves[h]], ones_sb[:, :], expT[j][:, halves[h]],
                            start=(j == 0), stop=(j == NT - 1),
                        )
                nc.vector.reciprocal_approx_fast(out=recipbc[:, :], in_=lps[:, :])

                # attn@V (contract over j), normalize on copy-out
                att = [qpool.tile([128, N], MMDT, tag="q", name="att") for _ in range(CT)]
                for d in range(CT):
                    ds_ = slice(128 * d, 128 * (d + 1))
                    aps = psA.tile([128, N], F32, tag="psA", name="psA")
                    for j in range(NT):
                        for h in range(NH):
                            nc.tensor.matmul(
                                aps[:, halves[h]], vT[j][:, ds_], expT[j][:, halves[h]],
                                start=(j == 0), stop=(j == NT - 1),
                            )
                    nc.vector.tensor_mul(att[d][:, :], aps[:, :], recipbc[:, :])
                s["att"] = att

            def emit_proj(s):
                att, xt, img = s["att"], s["xt"], s["img"]
                fin_all = fpool.tile([128, CT, N], F32, tag="f", name="f")
                for o in range(CT):
                    os_ = slice(128 * o, 128 * (o + 1))
                    pps = psA.tile([128, N], F32, tag="psA", name="psA")
                    for d in range(CT):
                        for h in range(NH):
                            nc.tensor.matmul(
                                pps[:, halves[h]], wp_all[:, d, os_], att[d][:, halves[h]],
                                start=(d == 0), stop=(d == CT - 1),
                            )
                    for h in range(NH):
                        nc.vector.tensor_add(
                            fin_all[:, o, halves[h]], pps[:, halves[h]],
                            xt[o].bitcast(F32)[:, halves[h]],
                        )
                        nc.sync.dma_start(
                            out=out_d[img, 128 * o:128 * (o + 1), halves[h]],
                            in_=fin_all[:, o, halves[h]],
                        )

            def _body():
                seq = [i % BPC for i in range(BPC * repeat)]
                states = [None] * len(seq)
                states[0] = emit_gn(seq[0], x_pre=x0 if seq[0] == 0 else None)
                emit_qkv(states[0])
                for i, img in enumerate(seq):
                    if i + 1 < len(seq):
                        states[i + 1] = emit_gn(seq[i + 1])
                    emit_att(states[i])
                    emit_proj(states[i])
                    states[i] = None
                    if i + 1 < len(seq):
                        emit_qkv(states[i + 1])

            if loop_iters:
                with tc.For_i(0, loop_iters, 1,
                              hint_engines=(mybir.EngineType.PE,
                                            mybir.EngineType.Activation,
                                            mybir.EngineType.DVE,
                                            mybir.EngineType.SP)):
                    _body()
            else:
                _body()

    nc.compile()
    return nc


def _prep_inputs(x, gn_scale, gn_bias, qkv_w, qkv_b, proj_w, proj_b,
                 attn_dt="f32r"):
    wdt = ml_dtypes.bfloat16 if attn_dt == "bf16" else np.float32
    f = np.float32
    x_r = np.asarray(x, dtype=f).reshape(B, C, N)
    qkv_w = np.asarray(qkv_w, dtype=f)
    qkv_b = np.asarray(qkv_b, dtype=f)
    proj_w = np.asarray(proj_w, dtype=f)
    proj_b = np.asarray(proj_b, dtype=f)
    # v-bias and proj-bias fold into a constant per-channel offset added to x
    # (rows of attn sum to 1): out += Wp @ bv + bp.
    bv = qkv_b[2 * C:3 * C]
    cvec = proj_w @ bv + proj_b
    if np.any(cvec):
        x_r = x_r + cvec[None, :, None]

    def col(v):
        return np.asarray(v, f).reshape(CT, 128).T

    consts = np.concatenate(
        [col(gn_scale), col(gn_bias), col(qkv_b[0:C] * SCALE), col(qkv_b[C:2 * C])],
        axis=1,
    )
    indicator = (np.arange(C)[:, None] // GS == np.arange(G)[None, :]).astype(f)
    common = {
        "wq": np.ascontiguousarray((qkv_w[0:C] * SCALE).T).astype(wdt),
        "wk": np.ascontiguousarray(qkv_w[C:2 * C].T).astype(wdt),
        "wv": np.ascontiguousarray(qkv_w[2 * C:3 * C].T).astype(wdt),
        "wp": np.ascontiguousarray(proj_w.T).astype(wdt),
        "ind": np.ascontiguousarray(indicator / (GS * N)),
        "bind": np.ascontiguousarray(indicator.T),
        "onesm": np.ones((128, 128), dtype=wdt),
        "consts": np.ascontiguousarray(consts),
    }
    in_maps = []
    for i in range(NCORES):
        m = dict(common)
        m["x"] = np.ascontiguousarray(x_r[BPC * i:BPC * (i + 1)])
        in_maps.append(m)
    return in_maps, not (np.any(qkv_b[0:C]) or np.any(qkv_b[C:2 * C]))


def kernel(x, gn_scale, gn_bias, qkv_w, qkv_b, proj_w, proj_b, _trace=False):
    in_maps, zero_qk = _prep_inputs(x, gn_scale, gn_bias, qkv_w, qkv_b,
                                    proj_w, proj_b, attn_dt=ATTN_DT)
    key = ("nc", zero_qk, ATTN_DT)
    if key not in _cache:
        _cache[key] = _build(
            zero_qk_bias=zero_qk,
            attn_dtype=BF16 if ATTN_DT == "bf16" else F32R,
        )
    nc = _cache[key]
    res = run_bass_kernel_spmd(nc, in_maps, core_ids=list(range(NCORES)),
                               trace=_trace)
    _cache["last_result"] = res
    out = np.stack([r["out"] for r in res.results], axis=0)
    return out.reshape(B, C, H, W)



# revision 4
# speedup vs baseline: 1.5261x; 1.5261x over previous
"""AttentionBlock (GroupNorm(32) + 1-head self-attention + proj + residual) on 8 trn2 cores.

Data-parallel over batch: each of the 8 NeuronCores processes 2 of the 16 images.
All large matmuls run in fp8-e4m3 with perf_mode=DoubleRow (2 MACs/cell/cycle:
each matmul contracts K=256 via paired k-tiles in a 3D [128, 2, free] AP).

Quantization scheme (tolerance is 2e-2; this lands ~1e-2):
  - Weights quantized host-side at Sw=32 (sigma ~1.4 in e4m3 normal range).
  - q stored as Sq*SCALE*(Wq@xn) (Sq=16), k,v as 1/Sw * psum; scores psum is
    then Sq*true_score and the exp activation applies scale=1/Sq, bias=-1.5.
    The -1.5 shift keeps exp <= ~60 (TRN e4m3 overflows to Inf above 240, no
    saturation) and cancels exactly in softmax.
  - att stored as Sa*(attn@V) (Sa=32); final proj copy-out folds 1/(Sw*Sa)
    into the residual add.
  - GroupNorm stats also via fp8 DoubleRow: indicator matmuls on x8/xsq8 with
    UNSCALED 1.0 indicator (1/16384 would denormal-flush in fp8); the mean
    scale is applied by the ACT reduce copies instead.

Layout strategy (per image, c=512 channels, n=1024 positions):
  - x f32r [128, CT, N]; xn/q/k/att fp8 [128, CT, N] (channel, position);
    vT fp8 [128, NT, C] (position, channel); expT fp8 [128, NT, N].
    Pair-tiles adjacent in the free dim enable the [:, 2u:2u+2, ...] DoubleRow
    slices for both stationary and moving operands.
  - softmax over j: no max subtraction (scores ~N(0,1)); column sums l[i] via
    an all-ones DoubleRow matmul which also broadcasts l across partitions.
  - rstd via Newton rsqrt on DVE; 1/l via reciprocal_approx_fast; ScalarE only
    needs one ACT table set (exp/copy/identity).
  - attn@V accumulates over j; normalization (and *Sa) folded into the
    PSUM->SBUF copy; proj scale + residual-add fused into the final DVE op.
  - DMAs batched; per-phase emission order software-pipelines the two images.
"""

import ml_dtypes
import numpy as np

import concourse.bacc as bacc
import concourse.tile as tile
import concourse.mybir as mybir
from concourse.bass_utils import run_bass_kernel_spmd

F32 = mybir.dt.float32
F32R = mybir.dt.float32r
BF16 = mybir.dt.bfloat16
FP8 = mybir.dt.float8e4
I32 = mybir.dt.int32
AF = mybir.ActivationFunctionType
ALU = mybir.AluOpType
AX = mybir.AxisListType
DR = mybir.MatmulPerfMode.DoubleRow
FP8NP = ml_dtypes.float8_e4m3

B, C, H, W = 16, 512, 32, 32
N = H * W                 # 1024 positions
NCORES = 8
BPC = B // NCORES         # 2 images per core
G = 32                    # groupnorm groups
GS = C // G               # 16 channels per group
CT = C // 128             # 4 channel tiles
NT = N // 128             # 8 position tiles
NH = N // 512             # 2 free-dim halves
EPS = 1e-5
SCALE = float(C) ** -0.5  # single head, head_dim = C
MAGIC = 0x5F3759DF        # Newton-rsqrt seed constant

SW = 32.0                 # weight quantization scale
SA = 32.0                 # att (attn@V) storage scale
SQ = 16.0                 # q storage prescale
EXPB = -1.5               # exp shift (cancels in softmax; keeps exp < 240)
QS = SCALE * SQ / SW      # q copy-out scale
KS = 1.0 / SW             # k / v copy-out scale
PS_ = 1.0 / (SW * SA)     # proj copy-out scale
INVGN = 1.0 / (GS * N)    # groupnorm mean scale (applied on ACT, not in fp8)

_cache: dict = {}

ATTN_DT = "fp8"


def _build(repeat: int = 1, zero_qk_bias: bool = True, loop_iters: int = 0,
           attn_dtype=None):
    nc = bacc.Bacc("TRN2", target_bir_lowering=False, num_devices=NCORES)

    x_d = nc.dram_tensor("x", [BPC, C, N], F32R, kind="ExternalInput")
    wq_d = nc.dram_tensor("wq", [C, C], FP8, kind="ExternalInput")     # (c, o) = Wq.T * Sw
    wk_d = nc.dram_tensor("wk", [C, C], FP8, kind="ExternalInput")     # Wk.T * Sw
    wv_d = nc.dram_tensor("wv", [C, C], FP8, kind="ExternalInput")     # Wv.T * Sw
    wp_d = nc.dram_tensor("wp", [C, C], FP8, kind="ExternalInput")     # Wp.T * Sw
    ind_d = nc.dram_tensor("ind", [C, G], FP8, kind="ExternalInput")   # 1.0 iff c//16==g
    bind_d = nc.dram_tensor("bind", [G, C], F32R, kind="ExternalInput")  # 0/1 indicator.T
    ones_d = nc.dram_tensor("onesm", [128, 2, 128], FP8, kind="ExternalInput")
    # consts: [gnsc | gnbi | bq*Sq*SCALE | bk], each (128, CT)
    consts_d = nc.dram_tensor("consts", [128, 4 * CT], F32, kind="ExternalInput")
    out_d = nc.dram_tensor("out", [BPC, C, N], F32, kind="ExternalOutput")

    with tile.TileContext(nc) as tc:
        with (
            tc.tile_pool(name="wpool", bufs=1) as wp_,
            tc.tile_pool(name="xpool", bufs=2) as xpool,
            tc.tile_pool(name="x8pool", bufs=2) as x8pool,
            tc.tile_pool(name="xnpool", bufs=2) as xnpool,
            tc.tile_pool(name="sqpool", bufs=1) as sqpool,
            tc.tile_pool(name="qpool", bufs=2) as qpool,
            tc.tile_pool(name="kpool", bufs=2) as kpool,
            tc.tile_pool(name="vpool", bufs=2) as vpool,
            tc.tile_pool(name="epool", bufs=2) as epool,
            tc.tile_pool(name="fpool", bufs=1) as fpool,
            tc.tile_pool(name="rpool", bufs=2) as rpool,
            tc.tile_pool(name="spool", bufs=2) as spool,
            tc.tile_pool(name="psA", bufs=3, space="PSUM") as psA,
            tc.tile_pool(name="psB", bufs=2, space="PSUM") as psB,
        ):
            # ---- persistent constants / weights (batched single DMAs) ----
            wq_all = wp_.tile([128, CT, C], FP8, tag="wq", name="wq")
            wk_all = wp_.tile([128, CT, C], FP8, tag="wk", name="wk")
            wv_all = wp_.tile([128, CT, C], FP8, tag="wv", name="wv")
            wp_all = wp_.tile([128, CT, C], FP8, tag="wp", name="wp")
            ind_all = wp_.tile([128, CT, G], FP8, tag="ind", name="ind")
            bind_all = wp_.tile([G, CT, 128], F32R, tag="bind", name="bind")
            ones_sb = wp_.tile([128, 2, 128], FP8, tag="ones", name="ones")
            consts_sb = wp_.tile([128, 4 * CT], F32, tag="consts", name="consts")
            magic_sb = wp_.tile([128, 1], I32, tag="magic", name="magic")
            nc.vector.memset(magic_sb, MAGIC)
            expb_sb = wp_.tile([128, 1], F32, tag="expb", name="expb")
            nc.vector.memset(expb_sb, EXPB)
            gnsc_sb = consts_sb[:, 0 * CT:1 * CT]
            gnbi_sb = consts_sb[:, 1 * CT:2 * CT]
            bq_sb = consts_sb[:, 2 * CT:3 * CT]
            bk_sb = consts_sb[:, 3 * CT:4 * CT]

            def part(dram2d):
                # (T*128, F) -> [128, T, F]
                return dram2d.rearrange("(t p) f -> p t f", p=128)

            # issue order matters: the first stats matmul needs only
            # ind + the first half of x tile 0 -- land those first
            x0 = xpool.tile([128, CT, N], F32R, tag="x", name="x")
            nc.sync.dma_start(out=x0[:, 0, 0:512], in_=x_d[0, 0:128, 0:512])
            nc.sync.dma_start(out=ind_all, in_=part(ind_d[:, :]))
            nc.sync.dma_start(out=x0[:, 0, 512:1024], in_=x_d[0, 0:128, 512:1024])
            for t in range(1, CT):
                nc.sync.dma_start(
                    out=x0[:, t, :], in_=x_d[0, 128 * t:128 * (t + 1), :]
                )
            nc.sync.dma_start(out=consts_sb, in_=consts_d[:, :])
            nc.sync.dma_start(
                out=bind_all, in_=bind_d.rearrange("g (t p) -> g t p", p=128)
            )
            nc.sync.dma_start(out=wq_all, in_=part(wq_d[:, :]))
            nc.sync.dma_start(out=wk_all, in_=part(wk_d[:, :]))
            nc.sync.dma_start(out=wv_all, in_=part(wv_d[:, :]))
            nc.sync.dma_start(out=ones_sb, in_=ones_d[:, :, :])
            nc.sync.dma_start(out=wp_all, in_=part(wp_d[:, :]))

            halves = [slice(0, 512), slice(512, 1024)]

            def emit_gn(img, x_pre=None):
                """Load x, GroupNorm stats + chain, xn apply (fp8 out).
                Returns state dict for later phases."""
                if x_pre is not None:
                    x_all = x_pre
                else:
                    x_all = xpool.tile([128, CT, N], F32R, tag="x", name="x")
                    for t in range(CT):
                        nc.sync.dma_start(
                            out=x_all[:, t, :],
                            in_=x_d[img, 128 * t:128 * (t + 1), :],
                        )
                xf = x_all.bitcast(F32)
                xt = [xf[:, t, :] for t in range(CT)]

                # x and x^2 in fp8 for the DoubleRow stats matmuls
                x8 = x8pool.tile([128, CT, N], FP8, tag="x8", name="x8")
                xsq8 = sqpool.tile([128, CT, N], FP8, tag="sq", name="sq")
                for t in range(CT):
                    nc.vector.tensor_copy(x8[:, t, :], xf[:, t, :])
                    nc.vector.tensor_mul(xsq8[:, t, :], xf[:, t, :], xf[:, t, :])

                # group sums via unscaled indicator DoubleRow matmuls
                sum_ps = psB.tile([G, 512], F32, tag="psB", name="psB")
                sum_ps2 = psB.tile([G, 512], F32, tag="psB", name="psB")
                for u in range(CT // 2):
                    for h, ps in ((0, sum_ps), (1, sum_ps2)):
                        nc.tensor.matmul(
                            ps[:, :], ind_all[:, 2 * u:2 * u + 2, :],
                            x8[:, 2 * u:2 * u + 2, halves[h]],
                            start=(u == 0), stop=(u == CT // 2 - 1),
                            perf_mode=DR,
                        )
                # reduce to st4 = [mean_h0, mean_h1, e2_h0, e2_h1] (ACT applies
                # the 1/16384 scale; recipbc reused as throwaway ACT output)
                st4 = spool.tile([G, 4], F32, tag="st4", name="st4")
                recipbc = rpool.tile([128, N], F32, tag="rbc", name="rbc")
                nc.scalar.activation(out=recipbc[0:G, 0:512], in_=sum_ps[:, :],
                                     func=AF.Copy, scale=INVGN,
                                     accum_out=st4[:, 0:1])
                nc.scalar.activation(out=recipbc[0:G, 512:1024], in_=sum_ps2[:, :],
                                     func=AF.Copy, scale=INVGN,
                                     accum_out=st4[:, 1:2])
                sq_ps = psB.tile([G, 512], F32, tag="psB", name="psB")
                sq_ps2 = psB.tile([G, 512], F32, tag="psB", name="psB")
                for u in range(CT // 2):
                    for h, ps in ((0, sq_ps), (1, sq_ps2)):
                        nc.tensor.matmul(
                            ps[:, :], ind_all[:, 2 * u:2 * u + 2, :],
                            xsq8[:, 2 * u:2 * u + 2, halves[h]],
                            start=(u == 0), stop=(u == CT // 2 - 1),
                            perf_mode=DR,
                        )
                nc.scalar.activation(out=recipbc[0:G, 0:512], in_=sq_ps[:, :],
                                     func=AF.Copy, scale=INVGN,
                                     accum_out=st4[:, 2:3])
                nc.scalar.activation(out=recipbc[0:G, 512:1024], in_=sq_ps2[:, :],
                                     func=AF.Copy, scale=INVGN,
                                     accum_out=st4[:, 3:4])
                # e12 = [mean, E[x^2]]
                e12 = spool.tile([G, 2], F32, tag="e12", name="e12")
                st4_v = st4.rearrange("g (s h) -> g s h", h=2)
                nc.vector.tensor_add(e12[:, :], st4_v[:, :, 0], st4_v[:, :, 1])
                # vpe = E2 + eps - mean^2
                vpe = spool.tile([G, 1], F32, tag="vpe", name="vpe")
                msq = spool.tile([G, 1], F32, tag="msq", name="msq")
                nc.vector.tensor_mul(msq[:, :], e12[:, 0:1], e12[:, 0:1])
                nc.vector.scalar_tensor_tensor(
                    out=vpe[:, :], in0=e12[:, 1:2], scalar=EPS, in1=msq[:, :],
                    op0=ALU.add, op1=ALU.subtract,
                )
                # rstd = 1/sqrt(vpe): bit-trick seed + 2 Newton iterations
                sh_t = spool.tile([G, 1], I32, tag="sh", name="sh")
                nc.vector.tensor_scalar(
                    out=sh_t[:, :], in0=vpe.bitcast(I32)[:, :], scalar1=1,
                    scalar2=None, op0=ALU.logical_shift_right,
                )
                seed = spool.tile([G, 1], I32, tag="seed", name="seed")
                nc.vector.scalar_tensor_tensor(
                    out=seed[:, :], in0=magic_sb[:G, :], scalar=0, in1=sh_t[:, :],
                    op0=ALU.bypass, op1=ALU.subtract,
                )
                y = seed.bitcast(F32)
                for it in range(2):
                    t1 = spool.tile([G, 1], F32, tag=f"nr{it}", name=f"nr{it}")
                    nc.vector.tensor_mul(t1[:, :], y[:, :], y[:, :])
                    nc.vector.tensor_mul(t1[:, :], t1[:, :], vpe[:, :])
                    nc.vector.tensor_scalar(
                        out=t1[:, :], in0=t1[:, :], scalar1=-0.5, scalar2=1.5,
                        op0=ALU.mult, op1=ALU.add,
                    )
                    y2 = spool.tile([G, 1], F32, tag=f"y{it}", name=f"y{it}")
                    nc.vector.tensor_mul(y2[:, :], y[:, :], t1[:, :])
                    y = y2
                # stats2 = [rstd, mean] (f32r for the broadcast matmul)
                stats2 = spool.tile([G, 2], F32R, tag="st2", name="st2")
                nc.vector.tensor_copy(stats2[:, 0:1], y[:, :])
                nc.vector.tensor_copy(stats2[:, 1:2], e12[:, 0:1])

                # broadcast to channels; a = gnsc*rstd, b = gnbi - mean*a
                bc_ps = psB.tile([128, 2 * CT], F32, tag="psB", name="psB")
                for t in range(CT):
                    nc.tensor.matmul(
                        bc_ps[:, 2 * t:2 * t + 2], bind_all[:, t, :], stats2[:, :],
                        start=True, stop=True,
                    )
                bc_sb = spool.tile([128, 2 * CT], F32, tag="bc", name="bc")
                nc.vector.tensor_copy(bc_sb[:, :], bc_ps[:, :])
                bc_v = bc_sb.rearrange("p (t s) -> p t s", s=2)
                a_all = spool.tile([128, CT], F32, tag="aall", name="aall")
                b_all = spool.tile([128, CT], F32, tag="ball", name="ball")
                nc.vector.tensor_mul(a_all[:, :], gnsc_sb, bc_v[:, :, 0])
                nc.vector.scalar_tensor_tensor(
                    out=b_all[:, :], in0=bc_v[:, :, 1], scalar=-1.0, in1=a_all[:, :],
                    op0=ALU.mult, op1=ALU.mult,
                )
                nc.vector.tensor_add(b_all[:, :], b_all[:, :], gnbi_sb)

                # xn = a*x + b -> fp8 (ACT; Identity is in every table set)
                xn = xnpool.tile([128, CT, N], FP8, tag="xn", name="xn")
                for t in range(CT):
                    nc.scalar.activation(
                        out=xn[:, t, :], in_=xt[t], func=AF.Identity,
                        scale=a_all[:, t:t + 1], bias=b_all[:, t:t + 1],
                    )
                return {"xt": xt, "xn": xn, "recipbc": recipbc, "img": img}

            def emit_qkv(s):
                xn = s["xn"]
                q = qpool.tile([128, CT, N], FP8, tag="q", name="q")
                k = kpool.tile([128, CT, N], FP8, tag="k", name="k")
                for d in range(CT):
                    ds_ = slice(128 * d, 128 * (d + 1))
                    qps = psA.tile([128, N], F32, tag="psA", name="psA")
                    for u in range(CT // 2):
                        for h in range(NH):
                            nc.tensor.matmul(
                                qps[:, halves[h]], wq_all[:, 2 * u:2 * u + 2, ds_],
                                xn[:, 2 * u:2 * u + 2, halves[h]],
                                start=(u == 0), stop=(u == CT // 2 - 1),
                                perf_mode=DR,
                            )
                    if zero_qk_bias:
                        nc.scalar.activation(out=q[:, d, :], in_=qps[:, :],
                                             func=AF.Copy, scale=QS)
                    else:
                        nc.scalar.activation(out=q[:, d, :], in_=qps[:, :],
                                             func=AF.Identity,
                                             bias=bq_sb[:, d:d + 1], scale=QS)
                    kps = psA.tile([128, N], F32, tag="psA", name="psA")
                    for u in range(CT // 2):
                        for h in range(NH):
                            nc.tensor.matmul(
                                kps[:, halves[h]], wk_all[:, 2 * u:2 * u + 2, ds_],
                                xn[:, 2 * u:2 * u + 2, halves[h]],
                                start=(u == 0), stop=(u == CT // 2 - 1),
                                perf_mode=DR,
                            )
                    if zero_qk_bias:
                        nc.scalar.activation(out=k[:, d, :], in_=kps[:, :],
                                             func=AF.Copy, scale=KS)
                    else:
                        nc.scalar.activation(out=k[:, d, :], in_=kps[:, :],
                                             func=AF.Identity,
                                             bias=bk_sb[:, d:d + 1], scale=KS)
                # vT[n, o]: lhsT = xn pair-slice, rhs = wv pair-slice
                vT = vpool.tile([128, NT, C], FP8, tag="v", name="v")
                for n in range(NT):
                    ns = slice(128 * n, 128 * (n + 1))
                    vps = psB.tile([128, 512], F32, tag="psB", name="psB")
                    for u in range(CT // 2):
                        nc.tensor.matmul(
                            vps[:, :], xn[:, 2 * u:2 * u + 2, ns],
                            wv_all[:, 2 * u:2 * u + 2, :],
                            start=(u == 0), stop=(u == CT // 2 - 1),
                            perf_mode=DR,
                        )
                    nc.vector.tensor_scalar(
                        out=vT[:, n, :], in0=vps[:, :], scalar1=KS,
                        scalar2=None, op0=ALU.mult,
                    )
                s["q"], s["k"], s["vT"] = q, k, vT

            def emit_att(s):
                q, k, vT, recipbc = s["q"], s["k"], s["vT"], s["recipbc"]
                expT = epool.tile([128, NT, N], FP8, tag="e", name="e")
                for j in range(NT):
                    js = slice(128 * j, 128 * (j + 1))
                    sps = psA.tile([128, N], F32, tag="psA", name="psA")
                    for u in range(CT // 2):
                        for h in range(NH):
                            nc.tensor.matmul(
                                sps[:, halves[h]], k[:, 2 * u:2 * u + 2, js],
                                q[:, 2 * u:2 * u + 2, halves[h]],
                                start=(u == 0), stop=(u == CT // 2 - 1),
                                perf_mode=DR,
                            )
                    # expT = exp(score/Sq + EXPB); shift cancels in softmax
                    nc.scalar.activation(out=expT[:, j, :], in_=sps[:, :],
                                         func=AF.Exp, scale=1.0 / SQ,
                                         bias=expb_sb[:, :])

                # l[i] = sum_j exp, broadcast across partitions; 1/l on DVE
                lps = psA.tile([128, N], F32, tag="psA", name="psA")
                for u in range(NT // 2):
                    for h in range(NH):
                        nc.tensor.matmul(
                            lps[:, halves[h]], ones_sb[:, :, :],
                            expT[:, 2 * u:2 * u + 2, halves[h]],
                            start=(u == 0), stop=(u == NT // 2 - 1),
                            perf_mode=DR,
                        )
                nc.vector.reciprocal_approx_fast(out=recipbc[:, :], in_=lps[:, :])

                # attn@V (contract over j), normalize*Sa on copy-out
                att = qpool.tile([128, CT, N], FP8, tag="q", name="att")
                for d in range(CT):
                    ds_ = slice(128 * d, 128 * (d + 1))
                    aps = psA.tile([128, N], F32, tag="psA", name="psA")
                    for u in range(NT // 2):
                        for h in range(NH):
                            nc.tensor.matmul(
                                aps[:, halves[h]], vT[:, 2 * u:2 * u + 2, ds_],
                                expT[:, 2 * u:2 * u + 2, halves[h]],
                                start=(u == 0), stop=(u == NT // 2 - 1),
                                perf_mode=DR,
                            )
                    nc.vector.scalar_tensor_tensor(
                        out=att[:, d, :], in0=aps[:, :], scalar=SA,
                        in1=recipbc[:, :], op0=ALU.mult, op1=ALU.mult,
                    )
                s["att"] = att

            def emit_proj(s):
                att, xt, img = s["att"], s["xt"], s["img"]
                fin_all = fpool.tile([128, CT, N], F32, tag="f", name="f")
                for o in range(CT):
                    os_ = slice(128 * o, 128 * (o + 1))
                    pps = psA.tile([128, N], F32, tag="psA", name="psA")
                    for u in range(CT // 2):
                        for h in range(NH):
                            nc.tensor.matmul(
                                pps[:, halves[h]], wp_all[:, 2 * u:2 * u + 2, os_],
                                att[:, 2 * u:2 * u + 2, halves[h]],
                                start=(u == 0), stop=(u == CT // 2 - 1),
                                perf_mode=DR,
                            )
                    for h in range(NH):
                        nc.vector.scalar_tensor_tensor(
                            out=fin_all[:, o, halves[h]], in0=pps[:, halves[h]],
                            scalar=PS_, in1=xt[o][:, halves[h]],
                            op0=ALU.mult, op1=ALU.add,
                        )
                        nc.sync.dma_start(
                            out=out_d[img, 128 * o:128 * (o + 1), halves[h]],
                            in_=fin_all[:, o, halves[h]],
                        )

            def _body():
                seq = [i % BPC for i in range(BPC * repeat)]
                states = [None] * len(seq)
                states[0] = emit_gn(seq[0], x_pre=x0 if seq[0] == 0 else None)
                emit_qkv(states[0])
                for i, img in enumerate(seq):
                    if i + 1 < len(seq):
                        states[i + 1] = emit_gn(seq[i + 1])
                    emit_att(states[i])
                    emit_proj(states[i])
                    states[i] = None
                    if i + 1 < len(seq):
                        emit_qkv(states[i + 1])

            if loop_iters:
                with tc.For_i(0, loop_iters, 1,
                              hint_engines=(mybir.EngineType.PE,
                                            mybir.EngineType.Activation,
                                            mybir.EngineType.DVE,
                                            mybir.EngineType.SP)):
                    _body()
            else:
                _body()

    nc.compile()
    return nc


def _prep_inputs(x, gn_scale, gn_bias, qkv_w, qkv_b, proj_w, proj_b,
                 attn_dt="fp8"):
    f = np.float32
    x_r = np.asarray(x, dtype=f).reshape(B, C, N)
    qkv_w = np.asarray(qkv_w, dtype=f)
    qkv_b = np.asarray(qkv_b, dtype=f)
    proj_w = np.asarray(proj_w, dtype=f)
    proj_b = np.asarray(proj_b, dtype=f)
    # v-bias and proj-bias fold into a constant per-channel offset added to x
    # (rows of attn sum to 1): out += Wp @ bv + bp.
    bv = qkv_b[2 * C:3 * C]
    cvec = proj_w @ bv + proj_b
    if np.any(cvec):
        x_r = x_r + cvec[None, :, None]

    def w8(m):
        return np.clip(np.ascontiguousarray(m.T) * SW, -240, 240).astype(FP8NP)

    def col(v):
        return np.asarray(v, f).reshape(CT, 128).T

    consts = np.concatenate(
        [col(gn_scale), col(gn_bias), col(qkv_b[0:C] * (SQ * SCALE)),
         col(qkv_b[C:2 * C])],
        axis=1,
    )
    indicator = (np.arange(C)[:, None] // GS == np.arange(G)[None, :]).astype(f)
    common = {
        "wq": w8(qkv_w[0:C]),
        "wk": w8(qkv_w[C:2 * C]),
        "wv": w8(qkv_w[2 * C:3 * C]),
        "wp": w8(proj_w),
        "ind": np.ascontiguousarray(indicator).astype(FP8NP),
        "bind": np.ascontiguousarray(indicator.T),
        "onesm": np.ones((128, 2, 128), dtype=FP8NP),
        "consts": np.ascontiguousarray(consts),
    }
    in_maps = []
    for i in range(NCORES):
        m = dict(common)
        m["x"] = np.ascontiguousarray(x_r[BPC * i:BPC * (i + 1)])
        in_maps.append(m)
    return in_maps, not (np.any(qkv_b[0:C]) or np.any(qkv_b[C:2 * C]))


def kernel(x, gn_scale, gn_bias, qkv_w, qkv_b, proj_w, proj_b, _trace=False):
    in_maps, zero_qk = _prep_inputs(x, gn_scale, gn_bias, qkv_w, qkv_b,
                                    proj_w, proj_b)
    key = ("nc", zero_qk)
    if key not in _cache:
        _cache[key] = _build(zero_qk_bias=zero_qk)
    nc = _cache[key]
    res = run_bass_kernel_spmd(nc, in_maps, core_ids=list(range(NCORES)),
                               trace=_trace)
    _cache["last_result"] = res
    out = np.stack([r["out"] for r in res.results], axis=0)
    return out.reshape(B, C, H, W)
